# revision 1
# baseline (speedup 1.0000x reference)
"""Chamfer-style point loss (nn_PointLoss) on 8 Trainium2 NeuronCores.

Math (reference): reflect points across plane n.x+d=0; half1 = reflected
points (valid where s=p.n+d < 0, mask m1), half2 = original points (mask
m2 = ~m1). D[i,j] = ||half1[i]-half2[j]||^2. Output scalar =
50*(sum_j min_i(D) m2_j / c2 + sum_i min_j(D) m1_i / c1).

v3 device formulation: F[i,j] = r1'[i] + r2'[j] - 2*a_i.b_j with penalty
P=2^14 on masked-out rows/cols. Row and column operand prep is merged
into one (128,36)-wide pass (cols appended as 4 extra columns) using
region-constant tiles; hi/lo bf16 splits write a single composite that is
scattered into K-major DRAM images with rearranged-destination DMAs
spread over the SP/Activation/Pool queues. One K=16 bf16 matmul per
(128,512) tile; tiles negated into fp16 so mins become maxes. Row-min
partials combined via one AllReduce(max) on a (128,33) f32 payload.

Sharding: half2 (column) axis split 8 ways, 512 cols/core; every core
holds all rows.
"""

import os
import sys

import numpy as np

for _p in ("/opt/trn_rl_repo", "/root/.axon_site/_ro/trn_rl_repo"):
    if os.path.isdir(_p) and _p not in sys.path:
        sys.path.insert(0, _p)

import concourse.bacc as bacc
import concourse.bass_isa as bass_isa
import concourse.tile as tile
from concourse import mybir
from concourse.bass_utils import run_bass_kernel_spmd

FP = mybir.dt.float32
BF = mybir.dt.bfloat16
HF = mybir.dt.float16
AX = mybir.AxisListType
OP = mybir.AluOpType

N = 4096
NCORES = 8
QT = 32            # row q-slots (q-inner layout: [p,q] is point 32p+q)
QC = QT // NCORES  # 4 col slots per partition (512 columns/core)
W = QT + QC        # merged row+col working width
PEN = float(2**14)  # keeps penalized F finite in fp16
BIG = 1.0e30
CMINIT = -60000.0


def _emit(tc, out_ap, norm_ap, pa_ap, oh_ap):
    nc = tc.nc

    psf = tc.alloc_tile_pool(name="psf", bufs=2, space="PSUM")
    pss = tc.alloc_tile_pool(name="pss", bufs=2, space="PSUM")
    per = tc.alloc_tile_pool(name="per", bufs=1)
    fsp = tc.alloc_tile_pool(name="fsp", bufs=3)
    drm = tc.alloc_tile_pool(name="drm", bufs=1, space="DRAM")

    def _t(shape, name, dt=FP):
        return per.tile(shape, dt, name=name)

    # ---- inputs to SBUF (split across both hwdge queues)
    norm_sb = _t([1, 4], "norm_sb")
    nc.sync.dma_start(norm_sb[:], norm_ap[:])
    PA = _t([128, W, 3], "PA")
    nc.sync.dma_start(PA[:], pa_ap[:])
    OH = _t([128, 1], "OH")
    nc.scalar.dma_start(OH[:], oh_ap[:])

    ones_c = _t([128, 1], "ones_c")
    nc.gpsimd.memset(ones_c[:], 1.0)
    ONES64 = _t([128, 64], "ONES64", BF)
    nc.gpsimd.memset(ONES64[:], 1.0)

    # region-constant tiles: first QT cols = row region, last QC = col region
    BETA = _t([128, W], "BETA")
    nc.gpsimd.memset(BETA[:, 0:QT], 1.0)
    nc.gpsimd.memset(BETA[:, QT:W], -2.0)
    SGNP = _t([128, W], "SGNP")
    nc.gpsimd.memset(SGNP[:, 0:QT], -PEN)
    nc.gpsimd.memset(SGNP[:, QT:W], PEN)
    ROWP = _t([128, W], "ROWP")
    nc.gpsimd.memset(ROWP[:, 0:QT], PEN)
    nc.gpsimd.memset(ROWP[:, QT:W], 0.0)
    RMASK = _t([128, W], "RMASK")
    nc.gpsimd.memset(RMASK[:, 0:QT], 1.0)
    nc.gpsimd.memset(RMASK[:, QT:W], 0.0)

    # ---- norm broadcast + plane constants
    NB = _t([128, 4], "NB")
    nc.gpsimd.partition_broadcast(NB[:], norm_sb[:], channels=128)
    nsq = _t([128, 4], "nsq")
    nc.vector.tensor_tensor(nsq[:], NB[:], NB[:], op=OP.mult)
    snn = _t([128, 1], "snn")
    nc.vector.tensor_reduce(snn[:], nsq[:, 0:3], axis=AX.X, op=OP.add)
    inv_nn = _t([128, 1], "inv_nn")
    nc.vector.reciprocal(inv_nn[:], snn[:])
    ninv2 = _t([128, 1], "ninv2")
    nc.scalar.mul(ninv2[:], inv_nn[:], -2.0)
    c4d = _t([128, 1], "c4d")
    nc.vector.tensor_tensor(c4d[:], NB[:, 3:4], inv_nn[:], op=OP.mult)
    nc.scalar.mul(c4d[:], c4d[:], 4.0)
    NINV2R = _t([128, W], "NINV2R")
    nc.vector.tensor_scalar(NINV2R[:], RMASK[:], ninv2[:], None, op0=OP.mult)
    C4DR = _t([128, W], "C4DR")
    nc.scalar.mul(C4DR[:], RMASK[:], c4d[:])

    # ---- merged plane eval: s = p.n + d, m1 = (s<0)
    s_all = _t([128, W], "s_all")
    t1_ = _t([128, W], "t1_")
    nc.scalar.mul(s_all[:], PA[:, :, 0], NB[:, 0:1])
    nc.scalar.mul(t1_[:], PA[:, :, 1], NB[:, 1:2])
    nc.vector.tensor_tensor(s_all[:], s_all[:], t1_[:], op=OP.add)
    nc.scalar.mul(t1_[:], PA[:, :, 2], NB[:, 2:3])
    nc.vector.tensor_tensor(s_all[:], s_all[:], t1_[:], op=OP.add)
    nc.vector.tensor_scalar_add(s_all[:], s_all[:], NB[:, 3:4])
    M1 = _t([128, W], "M1")
    nc.vector.tensor_scalar(M1[:], s_all[:], 0.0, None, op0=OP.is_lt)

    # ---- operand vectors V = alpha*n + beta.p (rows: reflect, cols: -2p)
    alpha = _t([128, W], "alpha")
    nc.vector.tensor_tensor(alpha[:], s_all[:], NINV2R[:], op=OP.mult)
    V = []
    for c in range(3):
        tv = _t([128, W], f"tv{c}")
        nc.scalar.mul(tv[:], alpha[:], NB[:, c : c + 1])
        tb2 = _t([128, W], f"tb2{c}")
        nc.gpsimd.tensor_tensor(tb2[:], BETA[:], PA[:, :, c], op=OP.mult)
        v = _t([128, W], f"v{c}")
        nc.vector.tensor_tensor(v[:], tv[:], tb2[:], op=OP.add)
        V.append(v)

    # ---- rr = |p|^2 + (4d/nn)*s*is_row + penalty
    pp = _t([128, W], "pp")
    nc.vector.tensor_tensor(pp[:], PA[:, :, 0], PA[:, :, 0], op=OP.mult)
    q1 = _t([128, W], "q1")
    nc.gpsimd.tensor_tensor(q1[:], PA[:, :, 1], PA[:, :, 1], op=OP.mult)
    nc.vector.tensor_tensor(pp[:], pp[:], q1[:], op=OP.add)
    q2 = _t([128, W], "q2")
    nc.gpsimd.tensor_tensor(q2[:], PA[:, :, 2], PA[:, :, 2], op=OP.mult)
    nc.vector.tensor_tensor(pp[:], pp[:], q2[:], op=OP.add)
    t3 = _t([128, W], "t3")
    nc.vector.tensor_tensor(t3[:], s_all[:], C4DR[:], op=OP.mult)
    t4 = _t([128, W], "t4")
    nc.gpsimd.tensor_tensor(t4[:], M1[:], SGNP[:], op=OP.mult)
    nc.gpsimd.tensor_tensor(t4[:], t4[:], ROWP[:], op=OP.add)
    rr = _t([128, W], "rr")
    nc.vector.tensor_tensor(rr[:], pp[:], t3[:], op=OP.add)
    nc.vector.tensor_tensor(rr[:], rr[:], t4[:], op=OP.add)

    # ---- bf16 hi/lo splits into one composite: slots [Vh0-2, Vl0-2, rrh, rrl]
    ACOMP = _t([128, 8, W], "ACOMP", BF)

    def split_into(src, hs, ls, name):
        nc.scalar.copy(ACOMP[:, hs, :], src[:])
        hf = _t([128, W], f"hf_{name}")
        nc.vector.tensor_scalar(hf[:], ACOMP[:, hs, :], 1.0, None, op0=OP.mult)
        lr = _t([128, W], f"lr_{name}")
        nc.gpsimd.tensor_tensor(lr[:], src[:], hf[:], op=OP.subtract)
        nc.scalar.copy(ACOMP[:, ls, :], lr[:])

    for c in range(3):
        split_into(V[c], c, 3 + c, f"v{c}")
    split_into(rr, 6, 7, "rr")

    # ---- K-major operand images via DRAM round trip (rearranged dests)
    stgA = drm.tile([16, 128, QT], BF, name="stgA")
    stgB = drm.tile([16, 128, QC], BF, name="stgB")

    AHI = ACOMP[:, 0:3, 0:QT]
    ALO = ACOMP[:, 3:6, 0:QT]
    ARR = ACOMP[:, 6:8, 0:QT]
    nc.sync.dma_start(stgA[0:3, :, :].rearrange("k i m -> i k m"), AHI)
    nc.scalar.dma_start(stgA[3:6, :, :].rearrange("k i m -> i k m"), AHI)
    nc.sync.dma_start(stgA[6:9, :, :].rearrange("k i m -> i k m"), ALO)
    nc.scalar.dma_start(stgA[9:12, :, :].rearrange("k i m -> i k m"), ALO)
    nc.sync.dma_start(stgA[12:14, :, :].rearrange("k i m -> i k m"), ARR)
    nc.gpsimd.dma_start(stgA[14:16, :, :], ONES64[:])

    BHI = ACOMP[:, 0:3, QT:W]
    BLO = ACOMP[:, 3:6, QT:W]
    BRR = ACOMP[:, 6:8, QT:W]
    nc.scalar.dma_start(stgB[0:3, :, :].rearrange("k p q -> p k q"), BHI)
    nc.gpsimd.dma_start(stgB[3:6, :, :].rearrange("k p q -> p k q"), BLO)
    nc.gpsimd.dma_start(stgB[6:9, :, :].rearrange("k p q -> p k q"), BHI)
    nc.gpsimd.dma_start(stgB[9:12, :, :].rearrange("k p q -> p k q"), BLO)
    nc.gpsimd.dma_start(stgB[12:14, :, :], ONES64[:, 0:8])
    nc.scalar.dma_start(stgB[14:16, :, :].rearrange("k p q -> p k q"), BRR)

    TA = _t([16, 128, QT], "TA", BF)
    nc.sync.dma_start(TA[:], stgA[:])
    TB = _t([16, 128, QC], "TB", BF)
    nc.scalar.dma_start(TB[:], stgB[:])

    # ---- masks: M1 tile-layout via DRAM reshuffle; m2 as (1,512) row
    stgQ = drm.tile([QT, 128], FP, name="stgQ")
    nc.gpsimd.dma_start(stgQ[:], M1[:, 0:QT])
    M1t = _t([128, QT], "M1t")
    nc.gpsimd.dma_start(M1t[:], stgQ[:].rearrange("g p -> p g"))
    M2CB = _t([128, QC], "M2CB")
    nc.vector.tensor_scalar(M2CB[:], M1[:, QT:W], -1.0, 1.0, op0=OP.mult, op1=OP.add)
    stgM = drm.tile([128, QC], FP, name="stgM")
    nc.gpsimd.dma_start(stgM[:], M2CB[:])
    M2row = _t([1, 512], "M2row")
    nc.gpsimd.dma_start(M2row[:], stgM[:])

    # ---- c1/c2 + reciprocals precomputed before the collective
    c1row = _t([128, 1], "c1row")
    nc.vector.tensor_reduce(c1row[:], M1[:, 0:QT], axis=AX.X, op=OP.add)
    c1_ps = pss.tile([1, 1], FP, tag="ps")
    nc.tensor.matmul(c1_ps[:], c1row[:], ones_c[:], start=True, stop=True)
    c1 = _t([1, 1], "c1")
    nc.scalar.copy(c1[:], c1_ps[:])
    c2 = _t([1, 1], "c2")
    nc.vector.tensor_scalar(c2[:], c1[:], -1.0, float(N), op0=OP.mult, op1=OP.add)
    nc.vector.tensor_scalar_max(c1[:], c1[:], 1.0)
    nc.vector.tensor_scalar_max(c2[:], c2[:], 1.0)
    rc1 = _t([1, 1], "rc1")
    nc.vector.reciprocal(rc1[:], c1[:])
    rc2 = _t([1, 1], "rc2")
    nc.vector.reciprocal(rc2[:], c2[:])

    # ---- main loop: FS = -(F tile) in fp16; row-max partials + col-max acc
    CM = _t([128, 512], "CM", HF)
    nc.gpsimd.memset(CM[:], CMINIT)
    D2 = _t([128, QT], "D2", HF)

    for m in range(QT):
        fps = psf.tile([128, 512], FP, name="fps")
        nc.tensor.matmul(
            fps[:], TA[:, 4 * m : 4 * (m + 1), :], TB[:], start=True, stop=True
        )
        FS = fsp.tile([128, 512], HF, name="FS")
        nc.scalar.mul(FS[:], fps[:], -1.0)
        nc.vector.tensor_reduce(D2[:, m : m + 1], FS[:], axis=AX.X, op=OP.max)
        nc.vector.tensor_tensor(CM[:], CM[:], FS[:], op=OP.max)

    # ---- columns: d1 = max over partitions, then masked sum s1
    D1B = _t([128, 512], "D1B")
    nc.gpsimd.partition_all_reduce(D1B[:], CM[:], 128, bass_isa.ReduceOp.max)
    w1 = _t([1, 512], "w1")
    nc.vector.tensor_tensor(w1[:], D1B[0:1, :], M2row[:], op=OP.mult)
    s1 = _t([1, 1], "s1")
    nc.vector.tensor_reduce(s1[:], w1[:], axis=AX.X, op=OP.add)

    # encode s1 into partition slot column: slot[core_id] = s1, else -BIG
    s1b = _t([128, 1], "s1b")
    nc.gpsimd.partition_broadcast(s1b[:], s1[:], channels=128)
    slot = _t([128, 1], "slot")
    nc.vector.tensor_tensor(slot[:], s1b[:], OH[:], op=OP.mult)
    bm = _t([128, 1], "bm")
    nc.vector.tensor_scalar(bm[:], OH[:], BIG, -BIG, op0=OP.mult, op1=OP.add)
    nc.vector.tensor_tensor(slot[:], slot[:], bm[:], op=OP.add)

    D2f = _t([128, QT], "D2f")
    nc.scalar.copy(D2f[:], D2[:])

    # ---- AllReduce(max) of [D2 | slot] over all 8 cores
    pay = drm.tile([128, QT + 1], FP, name="pay")
    pay2 = drm.tile([128, QT + 1], FP, name="pay2")
    nc.gpsimd.dma_start(pay[:, 0:QT], D2f[:])
    nc.gpsimd.dma_start(pay[:, QT : QT + 1], slot[:])
    nc.gpsimd.collective_compute(
        "AllReduce",
        OP.max,
        replica_groups=[list(range(NCORES))],
        ins=[pay.opt()],
        outs=[pay2.opt()],
    )
    G2 = _t([128, QT], "G2")
    nc.gpsimd.dma_start(G2[:], pay2[:, 0:QT])
    slots = _t([128, 1], "slots")
    nc.gpsimd.dma_start(slots[:], pay2[:, QT : QT + 1])

    # ---- finish: s2 = sum(G2*m1t), sum slots, combine with rc1/rc2
    w2 = _t([128, QT], "w2")
    nc.vector.tensor_tensor(w2[:], G2[:], M1t[:], op=OP.mult)
    w2s = _t([128, 1], "w2s")
    nc.vector.tensor_reduce(w2s[:], w2[:], axis=AX.X, op=OP.add)
    s2_ps = pss.tile([1, 1], FP, tag="ps")
    nc.tensor.matmul(s2_ps[:], w2s[:], ones_c[:], start=True, stop=True)

    sa_ps = pss.tile([1, 1], FP, tag="ps")
    nc.tensor.matmul(
        sa_ps[:], slots[0:NCORES, :], ones_c[0:NCORES, :], start=True, stop=True
    )

    s2 = _t([1, 1], "s2")
    nc.scalar.copy(s2[:], s2_ps[:])
    sum_s1 = _t([1, 1], "sum_s1")
    nc.scalar.copy(sum_s1[:], sa_ps[:])
    av2 = _t([1, 1], "av2")
    nc.vector.tensor_tensor(av2[:], s2[:], rc1[:], op=OP.mult)
    av1 = _t([1, 1], "av1")
    nc.vector.tensor_tensor(av1[:], sum_s1[:], rc2[:], op=OP.mult)
    res = _t([1, 1], "res")
    nc.vector.tensor_tensor(res[:], av1[:], av2[:], op=OP.add)
    nc.scalar.mul(res[:], res[:], -50.0)
    nc.sync.dma_start(out_ap[:], res[:])

    for p in (psf, pss, per, fsp, drm):
        p.seal()


_NC = None


def build():
    global _NC
    if _NC is not None:
        return _NC
    nc = bacc.Bacc(
        "TRN2", target_bir_lowering=False, debug=False, num_devices=NCORES
    )
    norm_ap = nc.dram_tensor("norm4", [1, 4], FP, kind="ExternalInput").ap()
    pa_ap = nc.dram_tensor("pa", [128, W, 3], FP, kind="ExternalInput").ap()
    oh_ap = nc.dram_tensor("oh", [128, 1], FP, kind="ExternalInput").ap()
    out_ap = nc.dram_tensor("out", [1, 1], FP, kind="ExternalOutput").ap()
    with tile.TileContext(nc) as tc:
        _emit(tc, out_ap, norm_ap, pa_ap, oh_ap)
    nc.compile()
    _NC = nc
    return nc


def make_in_maps(norm, points):
    norm = np.ascontiguousarray(norm, dtype=np.float32)
    pts = np.ascontiguousarray(points, dtype=np.float32)
    PTq = pts.reshape(128, QT, 3)
    maps = []
    for c in range(NCORES):
        oh = np.zeros((128, 1), np.float32)
        oh[c, 0] = 1.0
        cb = pts[512 * c : 512 * (c + 1)].reshape(128, QC, 3)
        pa = np.ascontiguousarray(np.concatenate([PTq, cb], axis=1))
        maps.append({"norm4": norm, "pa": pa, "oh": oh})
    return maps


LAST_RESULTS = None


def kernel(norm, points):
    global LAST_RESULTS
    nc = build()
    maps = make_in_maps(norm, points)
    trace = bool(os.environ.get("KERNEL_TRACE"))
    LAST_RESULTS = run_bass_kernel_spmd(
        nc, maps, list(range(NCORES)), trace=trace
    )
    out = np.asarray(LAST_RESULTS.results[0]["out"], dtype=np.float32)
    return out.reshape(())



# revision 23
# speedup vs baseline: 1.5073x; 1.5073x over previous
"""Chamfer-style point loss (nn_PointLoss) on 8 Trainium2 NeuronCores.

Math (reference): reflect points across plane n.x+d=0; half1 = reflected
points (valid where s=p.n+d < 0, mask m1), half2 = original points (mask
m2 = ~m1). D[i,j] = ||half1[i]-half2[j]||^2. Output scalar =
50*(sum_j min_i(D) m2_j / c2 + sum_i min_j(D) m1_i / c1).

v4 formulation: the reflection is affine, R = M.p + t with the
Householder matrix M = I - 2 n n^T / |n|^2 and t = -2d n/|n|^2, so
  F[i,j] = p_i . (-2 M p_j) + rowterm_i + colterm_j
with rowterm_i = |p_i|^2 + (4d/nn) s_i + PEN*(1-m1_i)
     colterm_j = |p_j|^2 + (4d/nn) s_j - 4d^2/nn + PEN*m1_j.
A-image K-rows 0..2 are the raw transposed points (host relayout);
row 3 = rowterm (one PE transpose + DRAM flatten); row 4 = ones.
B-image rows 0..2 = -2M.pcol (one 3x3 x 3x512 matmul); row 3 = ones;
row 4 = colterm. One K=5 fp32r matmul per (128,512) tile; mins taken
directly in min-space (fp16), col-min partition reduction via PE
transposes + batched X-reduce. Cross-core combine: AllReduce(min) on a
(128,33) fp16 payload with the s1 slot trick.

Sharding: half2 (column) axis split 8 ways, 512 cols/core; every core
holds all rows. Row index i=32p+q maps to A-column j=q*128+p (tile q,
partition p); col point 512c+4p+s maps to B-column 128s+p.
"""

import os
import sys

import numpy as np

for _p in ("/opt/trn_rl_repo", "/root/.axon_site/_ro/trn_rl_repo"):
    if os.path.isdir(_p) and _p not in sys.path:
        sys.path.insert(0, _p)

import concourse.bacc as bacc
import concourse.tile as tile
from concourse import mybir
from concourse.bass_utils import run_bass_kernel_spmd

FP = mybir.dt.float32
FR = mybir.dt.float32r
HF = mybir.dt.float16
AX = mybir.AxisListType
OP = mybir.AluOpType

N = 4096
NCORES = 8
QT = 32           # row tiles (tile m covers rows j in [128m, 128m+128))
QC = 4            # col slots per partition (512 columns/core)
W = QT + QC       # merged row+col working width in the [128, W] layout
PEN = float(2**14)
BIGH = 60000.0    # slot identity magnitude (max-space: -BIGH)
CMI = -60000.0    # CM init (max-space)


def _emit(tc, out_ap, norm_ap, pa_ap, aimg_ap, bimg_ap, pcol_ap, idf_ap,
          idh_ap, oh_ap, i2n_ap):
    nc = tc.nc

    psA = tc.alloc_tile_pool(name="psA", bufs=2, space="PSUM")
    psT = tc.alloc_tile_pool(name="psT", bufs=2, space="PSUM")
    per = tc.alloc_tile_pool(name="per", bufs=1)
    fsp = tc.alloc_tile_pool(name="fsp", bufs=2)
    drm = tc.alloc_tile_pool(name="drm", bufs=1, space="DRAM")

    def _t(shape, name, dt=FP):
        return per.tile(shape, dt, name=name)

    # ---- inputs to SBUF, spread across queues
    norm_sb = _t([1, 4], "norm_sb")
    nc.sync.dma_start(norm_sb[:], norm_ap[:])
    Aimg = _t([5, N], "Aimg", FR)
    nc.sync.dma_start(Aimg[:], aimg_ap[:].bitcast(FR))
    PA = _t([128, 3, W], "PA")
    nc.scalar.dma_start(PA[:], pa_ap[:])
    pcol = _t([3, 512], "pcol", FR)
    nc.scalar.dma_start(pcol[:], pcol_ap[:].bitcast(FR))
    Bimg = _t([5, 512], "Bimg", FR)
    nc.gpsimd.dma_start(Bimg[:], bimg_ap[:].bitcast(FR))
    OH = _t([128, 1], "OH")
    nc.gpsimd.dma_start(OH[:], oh_ap[:])
    IDF = _t([128, 128], "IDF")
    nc.sync.dma_start(IDF[:], idf_ap[:])
    IDH = _t([128, 128], "IDH", HF)
    nc.gpsimd.dma_start(IDH[:], idh_ap[:])

    # ---- early constants (off critical path)
    ones_r = _t([1, 128], "ones_r")
    nc.gpsimd.memset(ones_r[:], 1.0)
    ones_c = _t([128, 1], "ones_c")
    nc.gpsimd.memset(ones_c[:], 1.0)
    ones_ch = _t([128, 1], "ones_ch", HF)
    nc.gpsimd.memset(ones_ch[:], 1.0)
    SGNP = _t([128, W], "SGNP")
    nc.gpsimd.memset(SGNP[:, 0:QT], -PEN)
    nc.gpsimd.memset(SGNP[:, QT:W], PEN)
    ADDR = _t([128, W], "ADDR")
    nc.gpsimd.memset(ADDR[:, 0:QT], PEN)
    I2n = _t([3, 3], "I2n")
    nc.scalar.dma_start(I2n[:], i2n_ap[:])
    CM = _t([128, 512], "CM", HF)
    nc.gpsimd.memset(CM[:], CMI)

    # ---- norm-derived row [1,8]: [n0,n1,n2,d, 4d/nn, -4d^2/nn, 4/nn, _]
    nrow = _t([1, 8], "nrow")
    nc.gpsimd.memset(nrow[:], 0.0)
    nc.scalar.copy(nrow[:, 0:4], norm_sb[:])
    nsq = _t([1, 4], "nsq")
    nc.vector.tensor_tensor(nsq[:], norm_sb[:], norm_sb[:], op=OP.mult)
    nn_ = _t([1, 1], "nn_")
    nc.vector.tensor_reduce(nn_[:], nsq[:, 0:3], axis=AX.X, op=OP.add)
    inv_nn = _t([1, 1], "inv_nn")
    nc.vector.reciprocal(inv_nn[:], nn_[:])
    nc.vector.tensor_scalar(nrow[:, 6:7], inv_nn[:], 4.0, None, op0=OP.mult)
    nc.vector.tensor_tensor(nrow[:, 4:5], nrow[:, 6:7], norm_sb[:, 3:4],
                            op=OP.mult)
    t01 = _t([1, 1], "t01")
    nc.vector.tensor_scalar(t01[:], norm_sb[:, 3:4], -1.0, None, op0=OP.mult)
    nc.vector.tensor_tensor(nrow[:, 5:6], nrow[:, 4:5], t01[:], op=OP.mult)

    # broadcast to all 128 partitions via K=1 matmul
    nb_ps = psT.tile([128, 8], FP, name="nb_ps")
    nc.tensor.matmul(nb_ps[:], ones_r[:], nrow[:], start=True, stop=True)
    NB = _t([128, 8], "NB")
    nc.scalar.copy(NB[:], nb_ps[:])

    # ---- -2M = (4/nn) n n^T - 2I ; B3 = (-2M) @ pcol
    outer_ps = psT.tile([3, 3], FP, name="outer_ps")
    nc.tensor.matmul(outer_ps[:], norm_sb[:, 0:3], norm_sb[:, 0:3],
                     start=True, stop=True)
    statf = _t([3, 3], "statf")
    nc.vector.tensor_scalar(statf[:], outer_ps[:], NB[0:3, 6:7], None,
                            op0=OP.mult)
    stat = _t([3, 3], "stat", FR)
    nc.vector.tensor_tensor(stat[:], statf[:], I2n[:], op=OP.add)
    b3_ps = psT.tile([3, 512], FP, name="b3_ps")
    nc.tensor.matmul(b3_ps[:], stat[:], pcol[:], start=True, stop=True)
    nc.scalar.copy(Bimg[0:3, :], b3_ps[:])

    # ---- per-point chain in the [128, W] layout
    s_all = _t([128, W], "s_all")
    t1_ = _t([128, W], "t1_")
    nc.scalar.mul(s_all[:], PA[:, 0, :], NB[:, 0:1])
    nc.scalar.mul(t1_[:], PA[:, 1, :], NB[:, 1:2])
    nc.vector.tensor_tensor(s_all[:], s_all[:], t1_[:], op=OP.add)
    nc.scalar.mul(t1_[:], PA[:, 2, :], NB[:, 2:3])
    nc.vector.tensor_tensor(s_all[:], s_all[:], t1_[:], op=OP.add)
    nc.vector.tensor_scalar_add(s_all[:], s_all[:], NB[:, 3:4])

    pp = _t([128, W], "pp")
    nc.vector.tensor_tensor(pp[:], PA[:, 0, :], PA[:, 0, :], op=OP.mult)
    q1 = _t([128, W], "q1")
    nc.gpsimd.tensor_tensor(q1[:], PA[:, 1, :], PA[:, 1, :], op=OP.mult)
    nc.vector.tensor_tensor(pp[:], pp[:], q1[:], op=OP.add)
    q2 = _t([128, W], "q2")
    nc.gpsimd.tensor_tensor(q2[:], PA[:, 2, :], PA[:, 2, :], op=OP.mult)
    nc.vector.tensor_tensor(pp[:], pp[:], q2[:], op=OP.add)

    M1 = _t([128, W], "M1")
    nc.vector.tensor_scalar(M1[:], s_all[:], 0.0, None, op0=OP.is_lt)
    # ADDR col region = -4d^2/nn
    ones4 = _t([128, 4], "ones4")
    nc.gpsimd.memset(ones4[:], 1.0)
    nc.scalar.mul(ADDR[:, QT:W], ones4[:], NB[:, 5:6])

    # rowterm/colterm merged: rt = pp + (4d/nn) s + M1*SGNP + ADDR
    g_ = _t([128, W], "g_")
    nc.vector.tensor_scalar(g_[:], s_all[:], NB[:, 4:5], None, op0=OP.mult)
    nc.vector.tensor_tensor(g_[:], g_[:], pp[:], op=OP.add)
    t4 = _t([128, W], "t4")
    nc.gpsimd.tensor_tensor(t4[:], M1[:], SGNP[:], op=OP.mult)
    nc.vector.tensor_tensor(g_[:], g_[:], t4[:], op=OP.add)
    rt = _t([128, W], "rt")
    nc.vector.tensor_tensor(rt[:], g_[:], ADDR[:], op=OP.add)

    # masks for the tail (fp16), off critical path
    M1h = _t([128, QT], "M1h", HF)
    nc.scalar.copy(M1h[:], M1[:, 0:QT])

    # ---- transpose rt [128,36] -> [36,128]; flatten via DRAM
    rt_ps = psT.tile([W, 128], FP, name="rt_ps")
    nc.tensor.transpose(rt_ps[:], rt[:], IDF[:])
    rt_sb = _t([W, 128], "rt_sb")
    nc.scalar.copy(rt_sb[:], rt_ps[:])
    stg = drm.tile([W, 128], FP, name="stg")
    nc.sync.dma_start(stg[:], rt_sb[:])
    nc.gpsimd.dma_start(Aimg[3:4, :], stg[0:QT, :].bitcast(FR))
    nc.sync.dma_start(Bimg[4:5, :], stg[QT:W, :].bitcast(FR))

    # ---- c1/c2 + reciprocals (overlaps the main loop)
    c1row = _t([128, 1], "c1row")
    nc.vector.tensor_reduce(c1row[:], M1[:, 0:QT], axis=AX.X, op=OP.add)
    c1_ps = psT.tile([1, 1], FP, name="c1_ps")
    nc.tensor.matmul(c1_ps[:], c1row[:], ones_c[:], start=True, stop=True)
    c1 = _t([1, 1], "c1")
    nc.scalar.copy(c1[:], c1_ps[:])
    c2 = _t([1, 1], "c2")
    nc.vector.tensor_scalar(c2[:], c1[:], -1.0, float(N), op0=OP.mult,
                            op1=OP.add)
    nc.vector.tensor_scalar_max(c1[:], c1[:], 1.0)
    nc.vector.tensor_scalar_max(c2[:], c2[:], 1.0)
    rcv = _t([1, 2], "rcv")
    nc.vector.reciprocal(rcv[:, 0:1], c1[:])
    nc.vector.reciprocal(rcv[:, 1:2], c2[:])
    bm = _t([128, 1], "bm")
    nc.vector.tensor_scalar(bm[:], OH[:], BIGH, -BIGH, op0=OP.mult,
                            op1=OP.add)

    # ---- main loop: 16 batches of 2 row tiles
    Ar = Aimg[:]
    Br = Bimg[:]
    D2 = _t([128, QT], "D2")
    for b in range(QT // 2):
        ps = psA.tile([128, 2, 512], FP, name="ps")
        for t in range(2):
            m = 2 * b + t
            nc.tensor.matmul(ps[:, t, :], Ar[:, 128 * m : 128 * (m + 1)],
                             Br[:], start=True, stop=True)
        FS = fsp.tile([128, 2, 512], HF, name="FS")
        nc.scalar.mul(FS[:], ps[:], -1.0)
        nc.vector.tensor_reduce(D2[:, 2 * b : 2 * b + 2], FS[:], axis=AX.X,
                                op=OP.max)
        P = fsp.tile([128, 512], HF, tag="P", name="P")
        nc.vector.tensor_tensor(P[:], FS[:, 0, :], FS[:, 1, :], op=OP.max)
        nc.vector.tensor_tensor(CM[:], CM[:], P[:], op=OP.max)

    # ---- columns: transpose CM, batched X-reduce -> d1t [128,4]
    d1_ps = psT.tile([128, 4, 128], HF, tag="d", name="d1_ps")
    for g in range(4):
        nc.tensor.transpose(d1_ps[:, g, :], CM[:, 128 * g : 128 * (g + 1)],
                            IDH[:])
    d1t = _t([128, 4], "d1t", HF)
    nc.vector.tensor_reduce(d1t[:], d1_ps[:], axis=AX.X, op=OP.max)
    m2f = _t([128, QC], "m2f")
    nc.vector.tensor_scalar(m2f[:], M1[:, QT:W], -1.0, 1.0, op0=OP.mult,
                            op1=OP.add)
    m2fh = _t([128, QC], "m2fh", HF)
    nc.scalar.copy(m2fh[:], m2f[:])
    w1 = _t([128, 4], "w1", HF)
    nc.vector.tensor_tensor(w1[:], d1t[:], m2fh[:], op=OP.mult)
    w1s = _t([128, 1], "w1s")
    nc.vector.tensor_reduce(w1s[:], w1[:], axis=AX.X, op=OP.add)
    s1_ps = psT.tile([1, 1], FP, tag="t", name="s1_ps")
    nc.tensor.matmul(s1_ps[:], w1s[:], ones_c[:], start=True, stop=True)
    s1 = _t([1, 1], "s1")
    nc.scalar.copy(s1[:], s1_ps[:])

    # slot encode: pay[:,32] = s1 at partition core_id else +BIGH
    s1b_ps = psT.tile([128, 1], FP, tag="t", name="s1b_ps")
    nc.tensor.matmul(s1b_ps[:], ones_r[:], s1[:], start=True, stop=True)
    slot = _t([128, 1], "slot")
    nc.vector.tensor_tensor(slot[:], s1b_ps[:], OH[:], op=OP.mult)
    nc.vector.tensor_tensor(slot[:], slot[:], bm[:], op=OP.add)
    pay = _t([128, QT + 1], "pay", HF)
    nc.scalar.copy(pay[:, 0:QT], D2[:])
    nc.scalar.copy(pay[:, QT : QT + 1], slot[:])

    # ---- AllReduce(max) of [D2 | slot] over all 8 cores (fp16 payload)
    pd = drm.tile([128, QT + 1], HF, name="pd")
    pd2 = drm.tile([128, QT + 1], HF, name="pd2")
    nc.gpsimd.dma_start(pd[:], pay[:])
    nc.gpsimd.collective_compute(
        "AllReduce",
        OP.max,
        replica_groups=[list(range(NCORES))],
        ins=[pd.opt()],
        outs=[pd2.opt()],
    )
    G = _t([128, QT + 1], "G", HF)
    nc.gpsimd.dma_start(G[:], pd2[:])

    # ---- finish: s2 = sum(G2*m1h); sum slots; combine
    w2 = _t([128, QT], "w2", HF)
    nc.vector.tensor_tensor(w2[:], G[:, 0:QT], M1h[:], op=OP.mult)
    w2s = _t([128, 1], "w2s")
    nc.vector.tensor_reduce(w2s[:], w2[:], axis=AX.X, op=OP.add)
    s2_ps = psT.tile([1, 1], FP, name="s2_ps")
    nc.tensor.matmul(s2_ps[:], w2s[:], ones_c[:], start=True, stop=True)
    sa_ps = psT.tile([1, 1], FP, name="sa_ps")
    nc.tensor.matmul(sa_ps[:], G[0:NCORES, QT : QT + 1],
                     ones_ch[0:NCORES, :], start=True, stop=True)

    sv = _t([1, 2], "sv")
    nc.scalar.copy(sv[:, 0:1], s2_ps[:])
    nc.scalar.copy(sv[:, 1:2], sa_ps[:])
    pv = _t([1, 2], "pv")
    nc.vector.tensor_tensor(pv[:], sv[:], rcv[:], op=OP.mult)
    res = _t([1, 1], "res")
    nc.vector.tensor_reduce(res[:], pv[:], axis=AX.X, op=OP.add)
    nc.scalar.mul(res[:], res[:], -50.0)
    nc.sync.dma_start(out_ap[:], res[:])

    for p in (psA, psT, per, fsp, drm):
        p.seal()


_NC = None


def build():
    global _NC
    if _NC is not None:
        return _NC
    nc = bacc.Bacc(
        "TRN2", target_bir_lowering=False, debug=False, num_devices=NCORES
    )
    norm_ap = nc.dram_tensor("norm4", [1, 4], FP, kind="ExternalInput").ap()
    pa_ap = nc.dram_tensor("pa", [128, 3, W], FP, kind="ExternalInput").ap()
    aimg_ap = nc.dram_tensor("aimg", [5, N], FP, kind="ExternalInput").ap()
    bimg_ap = nc.dram_tensor("bimg", [5, 512], FP, kind="ExternalInput").ap()
    pcol_ap = nc.dram_tensor("pcol", [3, 512], FP, kind="ExternalInput").ap()
    idf_ap = nc.dram_tensor("idf", [128, 128], FP, kind="ExternalInput").ap()
    idh_ap = nc.dram_tensor("idh", [128, 128], HF, kind="ExternalInput").ap()
    oh_ap = nc.dram_tensor("oh", [128, 1], FP, kind="ExternalInput").ap()
    i2n_ap = nc.dram_tensor("i2n", [3, 3], FP, kind="ExternalInput").ap()
    out_ap = nc.dram_tensor("out", [1, 1], FP, kind="ExternalOutput").ap()
    with tile.TileContext(nc) as tc:
        _emit(tc, out_ap, norm_ap, pa_ap, aimg_ap, bimg_ap, pcol_ap, idf_ap,
              idh_ap, oh_ap, i2n_ap)
    nc.compile()
    _NC = nc
    return nc


def make_in_maps(norm, points):
    norm = np.ascontiguousarray(norm, dtype=np.float32)
    pts = np.ascontiguousarray(points, dtype=np.float32)
    # A-image: j = q*128 + p  <->  point id 32p+q
    ptsT = pts.reshape(128, QT, 3).transpose(1, 0, 2).reshape(N, 3).T
    aimg = np.zeros((5, N), np.float32)
    aimg[0:3] = ptsT
    aimg[4] = 1.0
    idf = np.eye(128, dtype=np.float32)
    idh = np.eye(128, dtype=np.float16)
    maps = []
    for c in range(NCORES):
        oh = np.zeros((128, 1), np.float32)
        oh[c, 0] = 1.0
        cb = pts[512 * c : 512 * (c + 1)].reshape(128, QC, 3)  # [p, s, 3]
        # pa: [p, comp, slot] with slots = 32 row slots + 4 col slots
        pa = np.concatenate(
            [pts.reshape(128, QT, 3), cb], axis=1
        ).transpose(0, 2, 1)  # [128, 3, 36]
        pa = np.ascontiguousarray(pa)
        # B columns: j = 128s + p  <->  col point 512c + 4p + s
        pcol = np.ascontiguousarray(
            cb.transpose(1, 0, 2).reshape(512, 3).T
        )  # [3, 512]
        bimg = np.zeros((5, 512), np.float32)
        bimg[3] = 1.0
        maps.append(
            {
                "norm4": norm,
                "pa": pa,
                "aimg": aimg,
                "bimg": bimg,
                "pcol": pcol,
                "idf": idf,
                "idh": idh,
                "oh": oh,
                "i2n": (-2.0 * np.eye(3)).astype(np.float32),
            }
        )
    return maps


LAST_RESULTS = None


def kernel(norm, points):
    global LAST_RESULTS
    nc = build()
    maps = make_in_maps(norm, points)
    trace = bool(os.environ.get("KERNEL_TRACE"))
    LAST_RESULTS = run_bass_kernel_spmd(
        nc, maps, list(range(NCORES)), trace=trace
    )
    out = np.asarray(LAST_RESULTS.results[0]["out"], dtype=np.float32)
    return out.reshape(())
